# revision 30
# baseline (speedup 1.0000x reference)
"""Cross-attention Trainium2 kernel (8 NeuronCores, Bass/Tile).

Problem (hardcoded): B=2, SQ=SKV=2048, D=1024, H=16 heads, HD=64.
  q  = query @ Wq + bq
  kv = context @ Wkv + bkv ; split into k, v per head
  o  = softmax(q k^T / sqrt(hd) + mask) v         (mask: -inf where True)
  out = o @ Wout + bout

Sharding: core c = (b, g) with b = c // 4 (batch), g = c % 4 (head group of 4).
Each core computes its batch's attention for its 4 heads and the partial out
projection (Wout rows for those heads); host sums the 4 partials per batch and
adds bout (linearity of the out projection).

Everything on-chip runs "transposed" (feature dim on partitions, tokens on the
free dim), so the host passes query/context transposed and gets the partial
output transposed back. Softmax uses no max subtraction (scores are ~N(0,1)
here; exp is safe in fp32) and folds masking into V: v rows are scaled by
keep=1-mask and an extra "keep" column of V yields the softmax denominator via
the same PE accumulation.

All matmuls run in float32r (fp32 with 11-bit mantissa, 4x faster than fp32 on
the PE). fp32r matmul inputs must be produced by rounding instructions or be
declared fp32r in DRAM; the host pre-rounds DRAM inputs (round-to-nearest-even
on 12 truncated bits, matching the on-chip cast).
"""

import sys

sys.path.insert(0, "/opt/trn_rl_repo")

import numpy as np

B, SQ, SKV, D, H, HD = 2, 2048, 2048, 1024, 16, 64
HG = 4                # heads per core
COLS = HG * HD        # 256 projected columns per core (per q/k/v)
DK = D // 128         # 8 contraction tiles
SQC = 512             # sq chunk (psum bank)
NSQC = SQ // SQC
SKC = 512             # skv chunk for kv projection
NSKC = SKV // SKC
NJ = SKV // 128       # 16 skv tiles for attention


def _round_fp32r(x: np.ndarray) -> np.ndarray:
    """Round fp32 to fp32r (drop 12 low mantissa bits, round-to-nearest-even)."""
    u = np.ascontiguousarray(x, dtype=np.float32).view(np.uint32)
    trunc = u & np.uint32(0xFFFFF000)
    rem = u & np.uint32(0xFFF)
    half = np.uint32(0x800)
    lsb = (u >> np.uint32(12)) & np.uint32(1)
    up = (rem > half) | ((rem == half) & (lsb == 1))
    return (trunc + (up.astype(np.uint32) << np.uint32(12))).view(np.float32)


_CACHE = {}


def _build(with_bias=False):
    import concourse.bacc as bacc
    import concourse.mybir as mybir
    import concourse.tile as tile

    F32 = mybir.dt.float32
    F32R = mybir.dt.float32r
    EXP = mybir.ActivationFunctionType.Exp

    nc = bacc.Bacc()

    # ---- DRAM I/O (per core) ----
    qryT = nc.dram_tensor("qryT", [D, SQ], F32R, kind="ExternalInput")
    ctxT = nc.dram_tensor("ctxT", [D, SKV], F32R, kind="ExternalInput")
    wq = nc.dram_tensor("wq", [D, COLS], F32R, kind="ExternalInput")
    wk = nc.dram_tensor("wk", [D, COLS], F32R, kind="ExternalInput")
    wv = nc.dram_tensor("wv", [D, COLS], F32R, kind="ExternalInput")
    wout = nc.dram_tensor("wout", [COLS, D], F32R, kind="ExternalInput")
    bq = nc.dram_tensor("bq", [1, COLS], F32R, kind="ExternalInput")
    bk = nc.dram_tensor("bk", [1, COLS], F32R, kind="ExternalInput")
    bv = nc.dram_tensor("bv", [1, COLS], F32R, kind="ExternalInput")
    ones = nc.dram_tensor("ones", [1, SQC], F32R, kind="ExternalInput")
    keep = nc.dram_tensor("keep", [128, NJ], F32, kind="ExternalInput")
    outT = nc.dram_tensor("outT", [D, SQ], F32, kind="ExternalOutput")

    with tile.TileContext(nc) as tc:
        with (
            tc.tile_pool(name="w", bufs=1) as wp,
            tc.tile_pool(name="big", bufs=1) as bigp,
            tc.tile_pool(name="strips", bufs=3) as sp,
            tc.tile_pool(name="work", bufs=1) as workp,
            tc.tile_pool(name="ps", bufs=1, space="PSUM") as psp,
        ):
            # ---- weights / constants ----
            # DMA order matters: the first context strip + phase-K weights
            # first so the PE can start ASAP; wq/wout later (phase A only).
            wq_sb = wp.tile([128, DK, COLS], F32R)
            wk_sb = wp.tile([128, DK, COLS], F32R)
            wv_sb = wp.tile([128, DK, COLS], F32R)
            wout_sb = wp.tile([128, 2, D], F32R)
            bq_sb = wp.tile([1, COLS], F32R)
            bk_sb = wp.tile([1, COLS], F32R)
            bv_sb = wp.tile([1, COLS], F32R)
            ones_sb = wp.tile([1, SQC], F32R)
            keep_sb = wp.tile([128, NJ], F32)

            ctxT_r = ctxT.ap().rearrange("(t p) s -> p t s", p=128)
            qryT_r = qryT.ap().rearrange("(t p) s -> p t s", p=128)
            outT_r = outT.ap().rearrange("(t p) s -> p t s", p=128)

            # Startup-critical DMAs split per d-tile so the first kT matmul
            # (needs wk d=0 + ctx d=0 only) starts after ~0.4 MB, not 3 MB.
            wk_r = wk.ap().rearrange("(t p) m -> p t m", p=128)
            ctx0_sb = sp.tile([128, DK, SKC], F32R, tag="strip")
            nc.sync.dma_start(wk_sb[:, 0:1, :], wk_r[:, 0:1, :])
            nc.sync.dma_start(ctx0_sb[:, 0:1, :], ctxT_r[:, 0:1, 0:SKC])
            nc.sync.dma_start(bk_sb[:], bk.ap())
            nc.sync.dma_start(ones_sb[:], ones.ap())
            for d in range(1, DK):
                nc.sync.dma_start(wk_sb[:, d:d + 1, :], wk_r[:, d:d + 1, :])
                nc.sync.dma_start(ctx0_sb[:, d:d + 1, :], ctxT_r[:, d:d + 1, 0:SKC])
            # qproj(0) runs between kT-jc0 and v-jc0 on the PE, so its
            # inputs (qry0, wq) come right after the kT inputs, split per
            # d-tile so its first matmul starts after ~0.4 MB.
            wq_r = wq.ap().rearrange("(t p) m -> p t m", p=128)
            qry0_sb = sp.tile([128, DK, SQC], F32R, tag="strip", name="qry0_sb")
            nc.sync.dma_start(bq_sb[:], bq.ap())
            for d in range(DK):
                nc.sync.dma_start(wq_sb[:, d:d + 1, :], wq_r[:, d:d + 1, :])
                nc.sync.dma_start(qry0_sb[:, d:d + 1, :], qryT_r[:, d:d + 1, 0:SQC])
            nc.sync.dma_start(wv_sb[:], wv.ap().rearrange("(t p) m -> p t m", p=128))
            nc.sync.dma_start(bv_sb[:], bv.ap())
            nc.sync.dma_start(keep_sb[:], keep.ap())
            # pre-issue the remaining ctx strips so they queue ahead of wout
            # and the later qry strips
            strip_tiles = [ctx0_sb]
            for jc in range(1, NSKC):
                st = sp.tile([128, DK, SKC], F32R, tag="strip", name=f"ctx{jc}_sb")
                nc.sync.dma_start(st[:], ctxT_r[:, :, jc * SKC:(jc + 1) * SKC])
                strip_tiles.append(st)

            # ---- persistent activations ----
            kt_sb = bigp.tile([128, 2, SKV], F32R)        # k^T, head pair per 64-row band
            v_sb = bigp.tile([128, NJ, HG, HD + 1], F32R)  # v + keep column, [skv%128, j, h, :]
            qt_all = bigp.tile([128, 2, SQ], F32R)         # q^T for all chunks

            # ============ Phase K as a generator (interleaved into head 0) ============
            # Each next() emits one skv chunk of the kv projection. Chunk 0's
            # attention for head 0 consumes kT/v columns in j order, so K-jc
            # sections are emitted just before the attention groups that need
            # them; the rest of chunk 0 + later chunks use qproj/outproj filler.

            def emit_K_kT(jc):
                ctx_sb = strip_tiles[jc]
                pk = psp.tile([128, 2, SKC], F32, tag="mm", bufs=2, name="pk")
                for cc in range(2):
                    for d in range(DK):
                        nc.tensor.matmul(
                            pk[:, cc, :],
                            wk_sb[:, d, cc * 128:(cc + 1) * 128],
                            ctx_sb[:, d, :],
                            start=(d == 0), stop=(not with_bias and d == DK - 1),
                        )
                    if with_bias:
                        nc.tensor.matmul(
                            pk[:, cc, :],
                            bk_sb[0:1, cc * 128:(cc + 1) * 128],
                            ones_sb[0:1, :],
                            start=False, stop=True,
                        )
                nc.vector.tensor_copy(kt_sb[:, :, jc * SKC:(jc + 1) * SKC], pk[:])

            def emit_K_v(jc):
                ctx_sb = strip_tiles[jc]
                for jjp in range(2):
                    pv = psp.tile([128, 2, SKC], F32, tag="mm", bufs=2, name="pv")
                    for sub in range(2):
                        jj = jjp * 2 + sub
                        for d in range(DK):
                            nc.tensor.matmul(
                                pv[:, sub, 0:COLS],
                                ctx_sb[:, d, jj * 128:(jj + 1) * 128],
                                wv_sb[:, d, :],
                                start=(d == 0), stop=(not with_bias and d == DK - 1),
                            )
                        if with_bias:
                            nc.tensor.matmul(
                                pv[:, sub, 0:COLS],
                                ones_sb[0:1, 0:128],
                                bv_sb[0:1, :],
                                start=False, stop=True,
                            )
                    for sub in range(2):
                        jj = jjp * 2 + sub
                        j = jc * 4 + jj
                        nc.vector.tensor_scalar_mul(
                            v_sb[:, j, :, 0:HD],
                            pv[:, sub, 0:COLS].rearrange("p (h e) -> p h e", h=HG),
                            keep_sb[:, j:j + 1],
                        )
                        for h in range(HG):
                            nc.vector.tensor_copy(
                                v_sb[:, j, h, HD:HD + 1], keep_sb[:, j:j + 1]
                            )

            def gen_phaseK_rest():
                for jc in range(1, NSKC):
                    if jc == NSKC - 1:
                        nc.sync.dma_start(wout_sb[:], wout.ap().rearrange("(t p) m -> p t m", p=128))
                    emit_K_kT(jc)
                    emit_K_v(jc)
                    yield

            # ====== Phase A: software-pipelined attention ======
            def gen_qproj(qc, qry_sb=None):
                if qry_sb is None:
                    qry_sb = sp.tile([128, DK, SQC], F32R, tag="strip", name="qry_sb")
                    nc.sync.dma_start(qry_sb[:], qryT_r[:, :, qc * SQC:(qc + 1) * SQC])
                yield
                pq = psp.tile([128, 2, SQC], F32, tag="fill", bufs=1, name="pq")
                for cc in range(2):
                    for d in range(DK):
                        nc.tensor.matmul(
                            pq[:, cc, :],
                            wq_sb[:, d, cc * 128:(cc + 1) * 128],
                            qry_sb[:, d, :],
                            start=(d == 0), stop=(not with_bias and d == DK - 1),
                        )
                        yield
                    if with_bias:
                        nc.tensor.matmul(
                            pq[:, cc, :],
                            bq_sb[0:1, cc * 128:(cc + 1) * 128],
                            ones_sb[0:1, :],
                            start=False, stop=True,
                        )
                    yield
                for cc in range(2):
                    nc.vector.tensor_copy(
                        qt_all[:, cc, qc * SQC:(qc + 1) * SQC], pq[:, cc, :]
                    )
                    yield

            def gen_outproj(qc, otn, tag="fill", nbufs=1):
                for mp in range(4):
                    pf = psp.tile([128, 2, SQC], F32, tag=tag, bufs=nbufs, name="pf")
                    for sub in range(2):
                        m = mp * 2 + sub
                        nc.tensor.matmul(
                            pf[:, sub, :],
                            wout_sb[:, 0, m * 128:(m + 1) * 128],
                            otn[:, 0, :],
                            start=True, stop=False,
                        )
                        yield
                        nc.tensor.matmul(
                            pf[:, sub, :],
                            wout_sb[:, 1, m * 128:(m + 1) * 128],
                            otn[:, 1, :],
                            start=False, stop=True,
                        )
                        yield
                    fin = workp.tile([128, 2, SQC], F32, tag="fin", bufs=4)
                    for sub in range(2):
                        m = mp * 2 + sub
                        nc.vector.tensor_copy(fin[:, sub, :], pf[:, sub, :])
                        nc.sync.dma_start(
                            outT_r[:, m:m + 1, qc * SQC:(qc + 1) * SQC],
                            fin[:, sub:sub + 1, :],
                        )
                        yield

            filler = []

            def emit_filler(budget):
                while budget > 0 and filler:
                    try:
                        next(filler[0])
                        budget -= 1
                    except StopIteration:
                        filler.pop(0)

            emit_K_kT(0)
            # chunk 0's q-projection runs between kT-jc0 and v-jc0
            for _ in gen_qproj(0, qry0_sb):
                pass
            emit_K_v(0)
            kgen = gen_phaseK_rest()

            otn_prev = None
            for qc in range(NSQC):
                if qc + 1 < NSQC:
                    filler.append(gen_qproj(qc + 1))
                if otn_prev is not None:
                    filler.append(gen_outproj(qc - 1, otn_prev))
                qt = qt_all[:, :, qc * SQC:(qc + 1) * SQC]
                otn = workp.tile([128, 2, SQC], F32R, tag="otn", bufs=2)
                for h in range(HG):
                    pair, po = h // 2, (h % 2) * 64
                    pav = psp.tile([HD + 1, SQC], F32, tag="av", bufs=2)
                    for jp in range(NJ // 2):
                        if qc == 0 and h == 0 and jp in (2, 4, 6):
                            next(kgen)  # emit K-jc before the groups needing it
                        ps = psp.tile([128, 2, SQC], F32, tag="mm", bufs=2)
                        for sub in range(2):
                            j = jp * 2 + sub
                            nc.tensor.matmul(
                                ps[:, sub, :],
                                kt_sb[po:po + 64, pair, j * 128:(j + 1) * 128],
                                qt[po:po + 64, pair, :],
                                start=True, stop=True,
                            )
                        pt = workp.tile([128, 2, SQC], F32R, tag="pt", bufs=3)
                        nc.scalar.activation(pt[:], ps[:], EXP)
                        for sub in range(2):
                            j = jp * 2 + sub
                            nc.tensor.matmul(
                                pav[:],
                                v_sb[:, j, h, :],
                                pt[:, sub, :],
                                start=(j == 0), stop=(j == NJ - 1),
                            )
                        if not (qc == 0 and h == 0):
                            emit_filler(2 if len(filler) > 1 else 1)
                    # normalize: divide by the keep-column accumulation
                    ot = workp.tile([HD + 1, SQC], F32, tag="ot", bufs=2)
                    nc.vector.tensor_copy(ot[:], pav[:])
                    rcp = workp.tile([1, SQC], F32R, tag="rcp", bufs=2)
                    with nc.allow_low_precision(reason="fp32r reciprocal for softmax denom"):
                        nc.vector.reciprocal(rcp[:], ot[HD:HD + 1, :])
                    emit_filler(1)
                    pbc = psp.tile([HD + 1, SQC], F32, tag="av", bufs=2)
                    nc.tensor.matmul(
                        pbc[0:HD, :], ones_sb[0:1, 0:HD], rcp[0:1, :],
                        start=True, stop=True,
                    )
                    nc.vector.tensor_mul(
                        otn[po:po + 64, pair, :], ot[0:HD, :], pbc[0:HD, :]
                    )
                otn_prev = otn

            # drain remaining filler, then the final chunk's out-projection
            emit_filler(10 ** 9)
            for _ in gen_outproj(NSQC - 1, otn_prev, tag="mm", nbufs=2):
                pass

    nc.compile()
    return nc


def _get_nc(with_bias=False):
    key = f"nc{int(with_bias)}"
    if key not in _CACHE:
        _CACHE[key] = _build(with_bias)
    return _CACHE[key]


LAST_RESULTS = None
LAST_IN_MAPS = None


def kernel(query, context, mask, Wq, bq, Wkv, bkv, Wout, bout, num_heads):
    import os
    from concourse.bass_utils import run_bass_kernel_spmd

    query = np.asarray(query, dtype=np.float32)
    context = np.asarray(context, dtype=np.float32)
    mask = np.asarray(mask)
    Wq = np.asarray(Wq, dtype=np.float32)
    bq_v = np.asarray(bq, dtype=np.float32)
    Wkv = np.asarray(Wkv, dtype=np.float32)
    bkv_v = np.asarray(bkv, dtype=np.float32)
    Wout = np.asarray(Wout, dtype=np.float32)
    bout_v = np.asarray(bout, dtype=np.float32)
    assert int(num_heads) == H

    scale = np.float32(HD ** -0.5)
    Wq_s = Wq * scale
    bq_s = bq_v * scale
    Wk = Wkv[:, :D]
    Wv = Wkv[:, D:]
    bk_v = bkv_v[:D]
    bv_v = bkv_v[D:]
    keep_f = 1.0 - mask.astype(np.float32)          # [B, SKV]
    ones_r = np.ones((1, SQC), dtype=np.float32)

    with_bias = bool(np.any(bq_s) or np.any(bk_v) or np.any(bv_v))
    nc = _get_nc(with_bias)
    in_maps = []
    for c in range(8):
        b, g = c // 4, c % 4
        cs = slice(g * COLS, (g + 1) * COLS)
        in_maps.append({
            "qryT": _round_fp32r(query[b].T),
            "ctxT": _round_fp32r(context[b].T),
            "wq": _round_fp32r(Wq_s[:, cs]),
            "wk": _round_fp32r(Wk[:, cs]),
            "wv": _round_fp32r(Wv[:, cs]),
            "wout": _round_fp32r(Wout[cs, :]),
            "bq": _round_fp32r(bq_s[cs][None, :]),
            "bk": _round_fp32r(bk_v[cs][None, :]),
            "bv": _round_fp32r(bv_v[cs][None, :]),
            "ones": ones_r,
            "keep": np.ascontiguousarray(keep_f[b].reshape(NJ, 128).T),
        })

    trace = bool(int(os.environ.get("KERNEL_TRACE", "0")))
    res = run_bass_kernel_spmd(nc, in_maps, core_ids=list(range(8)), trace=trace)
    global LAST_RESULTS, LAST_IN_MAPS
    LAST_RESULTS = res
    LAST_IN_MAPS = in_maps

    out = np.empty((B, SQ, D), dtype=np.float32)
    for b in range(B):
        acc = np.zeros((D, SQ), dtype=np.float32)
        for g in range(4):
            acc += res.results[b * 4 + g]["outT"]
        out[b] = acc.T + bout_v[None, :]
    return out


# revision 33
# speedup vs baseline: 11.7307x; 11.7307x over previous
"""Cross-attention Trainium2 kernel (8 NeuronCores, Bass/Tile).

Problem (hardcoded): B=2, SQ=SKV=2048, D=1024, H=16 heads, HD=64.
  q  = query @ Wq + bq
  kv = context @ Wkv + bkv ; split into k, v per head
  o  = softmax(q k^T / sqrt(hd) + mask) v         (mask: -inf where True)
  out = o @ Wout + bout

Sharding: core c = (b, g) with b = c // 4 (batch), g = c % 4 (head group of 4).
Each core computes its batch's attention for its 4 heads and the partial out
projection (Wout rows for those heads); host sums the 4 partials per batch and
adds bout (linearity of the out projection).

Everything on-chip runs "transposed" (feature dim on partitions, tokens on the
free dim), so the host passes query/context transposed and gets the partial
output transposed back. Softmax uses no max subtraction (scores are ~N(0,1)
here; exp is safe in fp32) and folds masking into V: v rows are scaled by
keep=1-mask and an extra "keep" column of V yields the softmax denominator via
the same PE accumulation.

All matmuls run in float32r (fp32 with 11-bit mantissa, 4x faster than fp32 on
the PE). fp32r matmul inputs must be produced by rounding instructions or be
declared fp32r in DRAM; the host pre-rounds DRAM inputs (round-to-nearest-even
on 12 truncated bits, matching the on-chip cast).
"""

import sys

sys.path.insert(0, "/opt/trn_rl_repo")

import numpy as np

B, SQ, SKV, D, H, HD = 2, 2048, 2048, 1024, 16, 64
HG = 4                # heads per core
COLS = HG * HD        # 256 projected columns per core (per q/k/v)
DK = D // 128         # 8 contraction tiles
SQC = 512             # sq chunk (psum bank)
NSQC = SQ // SQC
SKC = 512             # skv chunk for kv projection
NSKC = SKV // SKC
NJ = SKV // 128       # 16 skv tiles for attention


def _round_fp32r(x: np.ndarray) -> np.ndarray:
    """Round fp32 to fp32r (drop 12 low mantissa bits, round-to-nearest-even)."""
    u = np.ascontiguousarray(x, dtype=np.float32).view(np.uint32)
    trunc = u & np.uint32(0xFFFFF000)
    rem = u & np.uint32(0xFFF)
    half = np.uint32(0x800)
    lsb = (u >> np.uint32(12)) & np.uint32(1)
    up = (rem > half) | ((rem == half) & (lsb == 1))
    return (trunc + (up.astype(np.uint32) << np.uint32(12))).view(np.float32)


_CACHE = {}


def _build(with_bias=False):
    import concourse.bacc as bacc
    import concourse.mybir as mybir
    import concourse.tile as tile

    F32 = mybir.dt.float32
    F32R = mybir.dt.float32r
    EXP = mybir.ActivationFunctionType.Exp

    nc = bacc.Bacc()

    # ---- DRAM I/O (per core) ----
    qryT = nc.dram_tensor("qryT", [D, SQ], F32R, kind="ExternalInput")
    ctxT = nc.dram_tensor("ctxT", [D, SKV], F32R, kind="ExternalInput")
    wq = nc.dram_tensor("wq", [D, COLS], F32R, kind="ExternalInput")
    wk = nc.dram_tensor("wk", [D, COLS], F32R, kind="ExternalInput")
    wv = nc.dram_tensor("wv", [D, COLS], F32R, kind="ExternalInput")
    wout = nc.dram_tensor("wout", [COLS, D], F32R, kind="ExternalInput")
    bq = nc.dram_tensor("bq", [1, COLS], F32R, kind="ExternalInput")
    bk = nc.dram_tensor("bk", [1, COLS], F32R, kind="ExternalInput")
    bv = nc.dram_tensor("bv", [1, COLS], F32R, kind="ExternalInput")
    ones = nc.dram_tensor("ones", [1, SQC], F32R, kind="ExternalInput")
    keep = nc.dram_tensor("keep", [128, NJ], F32, kind="ExternalInput")
    outT = nc.dram_tensor("outT", [D, SQ], F32, kind="ExternalOutput")

    with tile.TileContext(nc) as tc:
        with (
            tc.tile_pool(name="w", bufs=1) as wp,
            tc.tile_pool(name="big", bufs=1) as bigp,
            tc.tile_pool(name="strips", bufs=3) as sp,
            tc.tile_pool(name="work", bufs=1) as workp,
            tc.tile_pool(name="ps", bufs=1, space="PSUM") as psp,
        ):
            # ---- weights / constants ----
            # DMA order matters: the first context strip + phase-K weights
            # first so the PE can start ASAP; wq/wout later (phase A only).
            wq_sb = wp.tile([128, DK, COLS], F32R)
            wk_sb = wp.tile([128, DK, COLS], F32R)
            wv_sb = wp.tile([128, DK, COLS], F32R)
            wout_sb = wp.tile([128, 2, D], F32R)
            bq_sb = wp.tile([1, COLS], F32R)
            bk_sb = wp.tile([1, COLS], F32R)
            bv_sb = wp.tile([1, COLS], F32R)
            ones_sb = wp.tile([1, SQC], F32R)
            keep_sb = wp.tile([128, NJ], F32)

            ctxT_r = ctxT.ap().rearrange("(t p) s -> p t s", p=128)
            qryT_r = qryT.ap().rearrange("(t p) s -> p t s", p=128)
            outT_r = outT.ap().rearrange("(t p) s -> p t s", p=128)

            # Startup-critical DMAs split per d-tile so the first kT matmul
            # (needs wk d=0 + ctx d=0 only) starts after ~0.4 MB, not 3 MB.
            wk_r = wk.ap().rearrange("(t p) m -> p t m", p=128)
            ctx0_sb = sp.tile([128, DK, SKC], F32R, tag="strip")
            nc.sync.dma_start(wk_sb[:, 0:1, :], wk_r[:, 0:1, :])
            nc.sync.dma_start(ctx0_sb[:, 0:1, :], ctxT_r[:, 0:1, 0:SKC])
            nc.sync.dma_start(bk_sb[:], bk.ap())
            nc.sync.dma_start(ones_sb[:], ones.ap())
            for d in range(1, DK):
                nc.sync.dma_start(wk_sb[:, d:d + 1, :], wk_r[:, d:d + 1, :])
                nc.sync.dma_start(ctx0_sb[:, d:d + 1, :], ctxT_r[:, d:d + 1, 0:SKC])
            # qproj(0) runs between kT-jc0 and v-jc0 on the PE, so its
            # inputs (qry0, wq) come right after the kT inputs, split per
            # d-tile so its first matmul starts after ~0.4 MB.
            wq_r = wq.ap().rearrange("(t p) m -> p t m", p=128)
            qry0_sb = sp.tile([128, DK, SQC], F32R, tag="strip", name="qry0_sb")
            nc.sync.dma_start(bq_sb[:], bq.ap())
            for d in range(DK):
                nc.sync.dma_start(wq_sb[:, d:d + 1, :], wq_r[:, d:d + 1, :])
                nc.sync.dma_start(qry0_sb[:, d:d + 1, :], qryT_r[:, d:d + 1, 0:SQC])
            nc.sync.dma_start(wv_sb[:], wv.ap().rearrange("(t p) m -> p t m", p=128))
            nc.sync.dma_start(bv_sb[:], bv.ap())
            nc.sync.dma_start(keep_sb[:], keep.ap())
            # pre-issue the remaining ctx strips so they queue ahead of wout
            # and the later qry strips
            strip_tiles = [ctx0_sb]
            for jc in range(1, NSKC):
                st = sp.tile([128, DK, SKC], F32R, tag="strip", name=f"ctx{jc}_sb")
                nc.sync.dma_start(st[:], ctxT_r[:, :, jc * SKC:(jc + 1) * SKC])
                strip_tiles.append(st)

            # ---- persistent activations ----
            kt_sb = bigp.tile([128, 2, SKV], F32R)        # k^T, head pair per 64-row band
            v_sb = bigp.tile([128, NJ, HG, HD + 1], F32R)  # v + keep column, [skv%128, j, h, :]
            qt_all = bigp.tile([128, 2, SQ], F32R)         # q^T for all chunks

            # ============ Phase K as a generator (interleaved into head 0) ============
            # Each next() emits one skv chunk of the kv projection. Chunk 0's
            # attention for head 0 consumes kT/v columns in j order, so K-jc
            # sections are emitted just before the attention groups that need
            # them; the rest of chunk 0 + later chunks use qproj/outproj filler.

            def emit_K_kT(jc):
                ctx_sb = strip_tiles[jc]
                pk = psp.tile([128, 2, SKC], F32, tag="mm", bufs=2, name="pk")
                for cc in range(2):
                    for d in range(DK):
                        nc.tensor.matmul(
                            pk[:, cc, :],
                            wk_sb[:, d, cc * 128:(cc + 1) * 128],
                            ctx_sb[:, d, :],
                            start=(d == 0), stop=(not with_bias and d == DK - 1),
                        )
                    if with_bias:
                        nc.tensor.matmul(
                            pk[:, cc, :],
                            bk_sb[0:1, cc * 128:(cc + 1) * 128],
                            ones_sb[0:1, :],
                            start=False, stop=True,
                        )
                nc.vector.tensor_copy(kt_sb[:, :, jc * SKC:(jc + 1) * SKC], pk[:])

            def emit_K_v(jc):
                ctx_sb = strip_tiles[jc]
                for jjp in range(2):
                    pv = psp.tile([128, 2, SKC], F32, tag="mm", bufs=2, name="pv")
                    for sub in range(2):
                        jj = jjp * 2 + sub
                        for d in range(DK):
                            nc.tensor.matmul(
                                pv[:, sub, 0:COLS],
                                ctx_sb[:, d, jj * 128:(jj + 1) * 128],
                                wv_sb[:, d, :],
                                start=(d == 0), stop=(not with_bias and d == DK - 1),
                            )
                        if with_bias:
                            nc.tensor.matmul(
                                pv[:, sub, 0:COLS],
                                ones_sb[0:1, 0:128],
                                bv_sb[0:1, :],
                                start=False, stop=True,
                            )
                    for sub in range(2):
                        jj = jjp * 2 + sub
                        j = jc * 4 + jj
                        nc.vector.tensor_scalar_mul(
                            v_sb[:, j, :, 0:HD],
                            pv[:, sub, 0:COLS].rearrange("p (h e) -> p h e", h=HG),
                            keep_sb[:, j:j + 1],
                        )
                        for h in range(HG):
                            nc.vector.tensor_copy(
                                v_sb[:, j, h, HD:HD + 1], keep_sb[:, j:j + 1]
                            )

            def gen_phaseK_rest():
                for jc in range(1, NSKC):
                    if jc == NSKC - 1:
                        nc.sync.dma_start(wout_sb[:], wout.ap().rearrange("(t p) m -> p t m", p=128))
                    emit_K_kT(jc)
                    emit_K_v(jc)
                    yield

            # ====== Phase A: software-pipelined attention ======
            def gen_qproj(qc, qry_sb=None):
                if qry_sb is None:
                    qry_sb = sp.tile([128, DK, SQC], F32R, tag="strip", name="qry_sb")
                    nc.sync.dma_start(qry_sb[:], qryT_r[:, :, qc * SQC:(qc + 1) * SQC])
                yield
                for cc in range(2):
                    pq = psp.tile([128, SQC], F32, tag="av", bufs=2, name="pq")
                    for d in range(DK):
                        nc.tensor.matmul(
                            pq[:],
                            wq_sb[:, d, cc * 128:(cc + 1) * 128],
                            qry_sb[:, d, :],
                            start=(d == 0), stop=(not with_bias and d == DK - 1),
                        )
                        yield
                    if with_bias:
                        nc.tensor.matmul(
                            pq[:],
                            bq_sb[0:1, cc * 128:(cc + 1) * 128],
                            ones_sb[0:1, :],
                            start=False, stop=True,
                        )
                        yield
                    nc.vector.tensor_copy(
                        qt_all[:, cc, qc * SQC:(qc + 1) * SQC], pq[:]
                    )
                    yield

            def gen_outproj(qc, otn):
                for m in range(8):
                    pf = psp.tile([128, SQC], F32, tag="av", bufs=2, name="pf")
                    nc.tensor.matmul(
                        pf[:],
                        wout_sb[:, 0, m * 128:(m + 1) * 128],
                        otn[:, 0, :],
                        start=True, stop=False,
                    )
                    yield
                    nc.tensor.matmul(
                        pf[:],
                        wout_sb[:, 1, m * 128:(m + 1) * 128],
                        otn[:, 1, :],
                        start=False, stop=True,
                    )
                    yield
                    fin = workp.tile([128, SQC], F32, tag="fin", bufs=4)
                    nc.vector.tensor_copy(fin[:], pf[:])
                    nc.sync.dma_start(
                        outT_r[:, m, qc * SQC:(qc + 1) * SQC], fin[:]
                    )
                    yield

            filler = []

            def emit_filler(budget):
                while budget > 0 and filler:
                    try:
                        next(filler[0])
                        budget -= 1
                    except StopIteration:
                        filler.pop(0)

            emit_K_kT(0)
            # chunk 0's q-projection runs between kT-jc0 and v-jc0
            for _ in gen_qproj(0, qry0_sb):
                pass
            emit_K_v(0)
            kgen = gen_phaseK_rest()

            otn_prev = None
            for qc in range(NSQC):
                if qc + 1 < NSQC:
                    filler.append(gen_qproj(qc + 1))
                if otn_prev is not None:
                    filler.append(gen_outproj(qc - 1, otn_prev))
                qt = qt_all[:, :, qc * SQC:(qc + 1) * SQC]
                otn = workp.tile([128, 2, SQC], F32R, tag="otn", bufs=2)
                for h in range(HG):
                    pair, po = h // 2, (h % 2) * 64
                    pav = psp.tile([HD + 1, SQC], F32, tag="av", bufs=2)

                    def emit_av(prev):
                        gs0, jbase0, pt0 = prev
                        for sub in range(gs0):
                            j = jbase0 + sub
                            nc.tensor.matmul(
                                pav[:],
                                v_sb[:, j, h, :],
                                pt0[:, sub, :],
                                start=(j == 0), stop=(j == NJ - 1),
                            )

                    # AV runs one group behind scores, so the PE never waits
                    # on a freshly issued exp.
                    prev = None
                    jbase = 0
                    for gi, gs in enumerate((3, 3, 3, 3, 2, 2)):
                        if qc == 0 and h == 0 and gi in (1, 2, 4):
                            next(kgen)  # emit K-jc before the groups needing it
                        ps = psp.tile([128, 3, SQC], F32, tag="mm", bufs=2)
                        for sub in range(gs):
                            j = jbase + sub
                            nc.tensor.matmul(
                                ps[:, sub, :],
                                kt_sb[po:po + 64, pair, j * 128:(j + 1) * 128],
                                qt[po:po + 64, pair, :],
                                start=True, stop=True,
                            )
                        pt = workp.tile([128, 3, SQC], F32R, tag="pt", bufs=3)
                        nc.scalar.activation(pt[:, 0:gs, :], ps[:, 0:gs, :], EXP)
                        if prev is not None:
                            emit_av(prev)
                        prev = (gs, jbase, pt)
                        jbase += gs
                        if not (qc == 0 and h == 0):
                            emit_filler(2 if len(filler) > 1 else 1)
                    emit_av(prev)
                    # normalize: divide by the keep-column accumulation
                    ot = workp.tile([HD + 1, SQC], F32, tag="ot", bufs=2)
                    nc.vector.tensor_copy(ot[:], pav[:])
                    rcp = workp.tile([1, SQC], F32R, tag="rcp", bufs=2)
                    with nc.allow_low_precision(reason="fp32r reciprocal for softmax denom"):
                        nc.vector.reciprocal(rcp[:], ot[HD:HD + 1, :])
                    emit_filler(1)
                    pbc = psp.tile([HD + 1, SQC], F32, tag="av", bufs=2)
                    nc.tensor.matmul(
                        pbc[0:HD, :], ones_sb[0:1, 0:HD], rcp[0:1, :],
                        start=True, stop=True,
                    )
                    nc.vector.tensor_mul(
                        otn[po:po + 64, pair, :], ot[0:HD, :], pbc[0:HD, :]
                    )
                otn_prev = otn

            # drain remaining filler, then the final chunk's out-projection
            emit_filler(10 ** 9)
            for _ in gen_outproj(NSQC - 1, otn_prev):
                pass

    nc.compile()
    return nc


def _get_nc(with_bias=False):
    key = f"nc{int(with_bias)}"
    if key not in _CACHE:
        _CACHE[key] = _build(with_bias)
    return _CACHE[key]


LAST_RESULTS = None
LAST_IN_MAPS = None


def kernel(query, context, mask, Wq, bq, Wkv, bkv, Wout, bout, num_heads):
    import os
    from concourse.bass_utils import run_bass_kernel_spmd

    query = np.asarray(query, dtype=np.float32)
    context = np.asarray(context, dtype=np.float32)
    mask = np.asarray(mask)
    Wq = np.asarray(Wq, dtype=np.float32)
    bq_v = np.asarray(bq, dtype=np.float32)
    Wkv = np.asarray(Wkv, dtype=np.float32)
    bkv_v = np.asarray(bkv, dtype=np.float32)
    Wout = np.asarray(Wout, dtype=np.float32)
    bout_v = np.asarray(bout, dtype=np.float32)
    assert int(num_heads) == H

    scale = np.float32(HD ** -0.5)
    Wq_s = Wq * scale
    bq_s = bq_v * scale
    Wk = Wkv[:, :D]
    Wv = Wkv[:, D:]
    bk_v = bkv_v[:D]
    bv_v = bkv_v[D:]
    keep_f = 1.0 - mask.astype(np.float32)          # [B, SKV]
    ones_r = np.ones((1, SQC), dtype=np.float32)

    with_bias = bool(np.any(bq_s) or np.any(bk_v) or np.any(bv_v))
    nc = _get_nc(with_bias)
    in_maps = []
    for c in range(8):
        b, g = c // 4, c % 4
        cs = slice(g * COLS, (g + 1) * COLS)
        in_maps.append({
            "qryT": _round_fp32r(query[b].T),
            "ctxT": _round_fp32r(context[b].T),
            "wq": _round_fp32r(Wq_s[:, cs]),
            "wk": _round_fp32r(Wk[:, cs]),
            "wv": _round_fp32r(Wv[:, cs]),
            "wout": _round_fp32r(Wout[cs, :]),
            "bq": _round_fp32r(bq_s[cs][None, :]),
            "bk": _round_fp32r(bk_v[cs][None, :]),
            "bv": _round_fp32r(bv_v[cs][None, :]),
            "ones": ones_r,
            "keep": np.ascontiguousarray(keep_f[b].reshape(NJ, 128).T),
        })

    trace = bool(int(os.environ.get("KERNEL_TRACE", "0")))
    res = run_bass_kernel_spmd(nc, in_maps, core_ids=list(range(8)), trace=trace)
    global LAST_RESULTS, LAST_IN_MAPS
    LAST_RESULTS = res
    LAST_IN_MAPS = in_maps

    out = np.empty((B, SQ, D), dtype=np.float32)
    for b in range(B):
        acc = np.zeros((D, SQ), dtype=np.float32)
        for g in range(4):
            acc += res.results[b * 4 + g]["outT"]
        out[b] = acc.T + bout_v[None, :]
    return out
